# revision 14
# baseline (speedup 1.0000x reference)
"""ChemConv Trainium2 kernel.

Computes, for A=2048 atoms, IN_DEPTH=D=128, OUT_DEPTH=O=128, FILTER_LEN=F=16:

  nc1[a,f,d]  = sum_b conn[a,b,f] * node[b,d]
  combined    = concat([nc1, bond], axis=2)            # (A, F, D+2)
  out[a,o]    = sum_{f,k} combined[a,f,k] * filters[o,f,k]

Sharding: atom rows of conn split across 8 NeuronCores (A/8 = 256 atoms each);
node/filters/bond replicated. No cross-core reduction.

The kernel is HBM-bound on the conn stream, so conn ships as bf16 (16.8MB/core
instead of 33.6MB; the b-contraction accumulates in fp32 PSUM, measured rel err
~2e-3 vs the 2e-2 gate). The host pre-packs conn into the exact SBUF layout the
matmuls consume -- per macro-block of 32 atoms: [bo=128 partitions][bi][a][f]
with b = bo*16 + bi -- so every DMA moves 16KB fully-contiguous per partition
and no on-chip reshuffle is needed. Filters/bond are host-pretransposed
(tiny), eliminating the PE transposes and the identity matrix of the fp32
version.

Per-core kernel:
  Stage 1 contracts b with bo on the 128 partitions and bi as 16
  PSUM-accumulated bf16 matmuls of free dim 512 (32 atoms x 16 f) per
  macro-block; PSUM (fp32) is copied to nc1[d, a, f] in SBUF as bf16.
  Stage 2 runs per half (128 atoms): one matmul per f against host-transposed
  filtT[d, f, o], plus one K=32 matmul for the bond term, accumulating
  out_T[o, a] in PSUM. Host transposes/concats the per-core (128, 256) outputs.
"""

import ml_dtypes
import numpy as np

import concourse.bacc as bacc
import concourse.mybir as mybir
import concourse.tile as tile
from concourse.bass_utils import run_bass_kernel_spmd

A, D, O, F = 2048, 128, 128, 16
NCORES = 8
AL = A // NCORES   # atoms per core = 256
MB = 16            # macro-blocks per core
ABK = AL // MB     # atoms per macro-block = 16
BO, BI = 128, 16   # b = bo*16 + bi

_f32 = mybir.dt.float32
_bf16 = mybir.dt.bfloat16
_f8 = mybir.dt.float8e3
_np_bf16 = ml_dtypes.bfloat16
_np_f8 = ml_dtypes.float8_e3m4


def _build():
    nc = bacc.Bacc("TRN2", target_bir_lowering=False, debug=False)

    conn = nc.dram_tensor("conn", [MB * BO, BI, ABK * F], _f8, kind="ExternalInput")
    node = nc.dram_tensor("node", [BO, BI * D], _bf16, kind="ExternalInput")
    filtT = nc.dram_tensor("filtT", [D, F * O], _bf16, kind="ExternalInput")
    bfiltT = nc.dram_tensor("bfiltT", [F * 2, O], _bf16, kind="ExternalInput")
    bondT = nc.dram_tensor("bondT", [F * 2, AL], _bf16, kind="ExternalInput")
    out = nc.dram_tensor("out", [O, AL], _f32, kind="ExternalOutput")

    with tile.TileContext(nc) as tc:
        with (
            tc.tile_pool(name="sb", bufs=1) as sb,
            tc.tile_pool(name="connp", bufs=6) as connp,
            tc.tile_pool(name="ps1", bufs=4, space="PSUM") as ps1,
            tc.tile_pool(name="ps2", bufs=1, space="PSUM") as ps2,
        ):
            # node rides first on the sync ring (every stage-1 matmul needs
            # it), the first conn blocks follow; filtT/bond ride the scalar
            # ring behind ct1. The bi=0/1 slices of node and ct0 go as tiny
            # head DMAs so the first matmuls can start ~2us earlier.
            node_sb = sb.tile([BO, BI * D], _bf16)
            nc.sync.dma_start(node_sb[:, 0:D], node[:, 0:D])
            ct_pre = []
            for mb in range(2):
                ct = connp.tile([BO, BI, ABK * F], _f8, tag="conn")
                eng = nc.scalar if mb % 2 == 0 else nc.sync
                if mb == 0:
                    eng.dma_start(ct[:, 0:2, :], conn[0:BO, 0:2, :])
                    eng.dma_start(ct[:, 2:, :], conn[0:BO, 2:, :])
                else:
                    eng.dma_start(ct[:], conn[mb * BO : (mb + 1) * BO])
                ct_pre.append(ct)
            nc.sync.dma_start(node_sb[:, D:], node[:, D:])
            filtT_sb = sb.tile([D, F * O], _bf16)
            nc.scalar.dma_start(filtT_sb[:], filtT[:])
            bfiltT_sb = sb.tile([F * 2, O], _bf16)
            nc.scalar.dma_start(bfiltT_sb[:], bfiltT[:])
            bondT_sb = sb.tile([F * 2, AL], _bf16)
            nc.scalar.dma_start(bondT_sb[:], bondT[:])

            # Stage 1: nc1[d, f, a] = sum_b node[b, d] * conn[a, b, f]
            # (f-major so stage-2 rhs slices are contiguous). Stage 2 runs per
            # half (atoms 0:128 / 128:256) as soon as that half's blocks are
            # done, so only the second half sits in the tail.
            nc1_sb = sb.tile([D, F, AL], _bf16)
            out_sb = sb.tile([O, AL], _f32)

            def stage2_half(h):
                a0 = h * (AL // 2)
                p2 = ps2.tile([O, AL // 2], _f32, tag="p2")
                for f in range(F):
                    nc.tensor.matmul(
                        p2[:],
                        filtT_sb[:, f * O : (f + 1) * O],
                        nc1_sb[:, f, a0 : a0 + AL // 2],
                        start=(f == 0),
                        stop=False,
                    )
                nc.tensor.matmul(
                    p2[:],
                    bfiltT_sb[:],
                    bondT_sb[:, a0 : a0 + AL // 2],
                    start=False,
                    stop=True,
                )
                nc.vector.tensor_copy(out_sb[:, a0 : a0 + AL // 2], p2[:])
                nc.scalar.dma_start(out[:, a0 : a0 + AL // 2], out_sb[:, a0 : a0 + AL // 2])

            for mb in range(MB):
                if mb < 2:
                    ct = ct_pre[mb]
                else:
                    ct = connp.tile([BO, BI, ABK * F], _f8, tag="conn")
                    eng = nc.scalar if mb % 2 == 0 else nc.sync
                    eng.dma_start(ct[:], conn[mb * BO : (mb + 1) * BO])
                p1 = ps1.tile([D, ABK * F], _f32, tag="p1")
                for bi in range(BI):
                    nc.tensor.matmul(
                        p1[:],
                        node_sb[:, bi * D : (bi + 1) * D],
                        ct[:, bi, :],
                        start=(bi == 0),
                        stop=(bi == BI - 1),
                    )
                nc.vector.tensor_copy(
                    nc1_sb[:, :, mb * ABK : (mb + 1) * ABK],
                    p1[:].rearrange("p (f a) -> p f a", a=ABK),
                )
                if mb == MB // 2 - 1:
                    stage2_half(0)
            stage2_half(1)

    nc.compile()
    return nc


def _in_maps(node_property_tensor, connectivity_tensor, bond_property_tensor, filters):
    node = np.asarray(node_property_tensor, dtype=np.float32)
    conn = np.asarray(connectivity_tensor, dtype=np.float32)
    bond = np.asarray(bond_property_tensor, dtype=np.float32)
    filt = np.asarray(filters, dtype=np.float32)

    node_p = np.ascontiguousarray(node.reshape(BO, BI * D)).astype(_np_bf16)
    # filters[o, f, :D] -> filtT[d, (f o)]
    filtT = np.ascontiguousarray(filt[:, :, :D].transpose(2, 1, 0)).astype(
        _np_bf16
    ).reshape(D, F * O)
    # filters[o, f, D:D+2] -> bfiltT[(f j), o]
    bfiltT = np.ascontiguousarray(filt[:, :, D:].transpose(1, 2, 0)).astype(
        _np_bf16
    ).reshape(F * 2, O)

    conn_q = conn.astype(_np_f8)
    maps = []
    for c in range(NCORES):
        cs = conn_q[c * AL : (c + 1) * AL]  # (AL, B=2048, F)
        # pack [mb, bo, bi, f, a]: f-major per bi so stage-1 PSUM columns come
        # out (f, a) and stage-2 rhs slices are contiguous
        cp = np.ascontiguousarray(
            cs.reshape(MB, ABK, BO, BI, F).transpose(0, 2, 3, 4, 1)
        ).reshape(MB * BO, BI, ABK * F)
        bs = bond[c * AL : (c + 1) * AL]  # (AL, F, 2)
        bT = np.ascontiguousarray(bs.transpose(1, 2, 0)).astype(_np_bf16).reshape(
            F * 2, AL
        )
        maps.append(
            {
                "conn": cp,
                "node": node_p,
                "filtT": filtT,
                "bfiltT": bfiltT,
                "bondT": bT,
            }
        )
    return maps


def _enable_tracing():
    """Install the NTFF profile hook (missing antenv.axon_hooks shim) and
    neuter the artifact upload (zero-egress container). Profiling only —
    never touched on the plain kernel() path."""
    import sys
    import types

    try:
        import antenv.axon_hooks  # noqa: F401
    except ImportError:
        from trn_agent_boot.trn_boot import _ntff_profile_via_ctypes

        hook = _ntff_profile_via_ctypes("/opt/axon/libaxon_pjrt.so")
        mod = types.ModuleType("antenv.axon_hooks")
        mod._hook = hook
        mod.get_axon_ntff_profile_hook = lambda: mod._hook
        mod.set_axon_ntff_profile_hook = lambda h: setattr(mod, "_hook", h)
        sys.modules["antenv.axon_hooks"] = mod
        import antenv

        antenv.axon_hooks = mod

    import concourse.bass_utils as _bu

    _bu.upload_artifacts = lambda tmpdir: tmpdir


def run(
    node_property_tensor,
    connectivity_tensor,
    bond_property_tensor,
    filters,
    trace=False,
):
    """Run the sharded kernel; returns (full (A, O) output, exec_time_ns|None)."""
    if trace:
        _enable_tracing()
    nc = _build()
    maps = _in_maps(
        node_property_tensor, connectivity_tensor, bond_property_tensor, filters
    )
    res = run_bass_kernel_spmd(nc, maps, core_ids=list(range(NCORES)), trace=trace)
    parts = [res.results[c]["out"] for c in range(NCORES)]  # each (O, AL)
    full = np.concatenate(parts, axis=1).T  # (A, O)
    return np.ascontiguousarray(full, dtype=np.float32), res.exec_time_ns


def kernel(
    node_property_tensor, connectivity_tensor, bond_property_tensor, filters
) -> np.ndarray:
    out, _ = run(
        node_property_tensor, connectivity_tensor, bond_property_tensor, filters
    )
    return out
